# revision 4
# baseline (speedup 1.0000x reference)
"""BertSelfAttention on 8 Trainium2 NeuronCores (Bass/Tile).

Sharding: data-parallel over batch (B=2) x tensor-parallel over heads
(16 heads -> 4 groups of 4). Core c handles batch c//4, head group c%4,
holding column shards of Wq/Wk/Wv. No collectives.

Per-core math (S=2048, D_group=256, 4 heads of d=64):
  xT   [1024, 2048]  (host-pretransposed hidden states for this batch)
  QT   = Wq_g^T @ x^T + bq_g      [256, 2048]   (d' on partitions)
  KT   = Wk_g^T @ x^T + bk_g      [256, 2048]
  V    = x @ Wv_g + bv_g          [2048, 260]   (head-major, 65 cols/head:
                                                 64 V dims + ones column)
  per head h, k-tile kt, q-chunk qc:
    S^T[k,q]  = sum_d KT[d,k] QT[d,q]      (row-packed pairs: 2 heads fill
                                            the 128-row contraction)
    E = exp(S^T / 8)                       (ScalarE, reads PSUM directly)
    ctx^T[r,q] += sum_k Vaug[k,r] E[k,q]   (r=0..63 context, r=64 softmax
                                            denominator via ones column)
  out_raw [260, 2048] = 4 heads x (64 ctx rows + 1 sums row)

Host unshards: out[b, :, g*256 + 64h + r] = (ctx_h / sums_h).T
"""

import sys

sys.path.insert(0, "/opt/trn_rl_repo")

import numpy as np

import concourse.bass as bass
import concourse.mybir as mybir
import concourse.tile as tile
from concourse import bacc
from concourse.bass_utils import run_bass_kernel_spmd

F32 = mybir.dt.float32

HIDDEN = 1024
NUM_HEADS = 16
HEAD = 64
B, S = 2, 2048
N_CORES = 8
GROUPS = 4                      # head groups (tensor parallel)
HG = NUM_HEADS // GROUPS        # heads per group = 4
DG = HG * HEAD                  # 256 cols per group
KT_TILES = HIDDEN // 128        # 8 contraction tiles for projections
ST_TILES = S // 128             # 16 sequence tiles
QC = 512                        # q chunk width
N_QC = S // QC                  # 4
VAUG = HG * (HEAD + 1)          # 260: [V_h | ones] per head


def _build_kernel():
    nc = bacc.Bacc("TRN2")

    xT = nc.dram_tensor("xT", [HIDDEN, S], F32, kind="ExternalInput")
    wq = nc.dram_tensor("wq", [HIDDEN, DG], F32, kind="ExternalInput")
    wk = nc.dram_tensor("wk", [HIDDEN, DG], F32, kind="ExternalInput")
    # wv pre-augmented on host: per head 64 cols + a zero col -> [1024, 260]
    wv = nc.dram_tensor("wv", [HIDDEN, VAUG], F32, kind="ExternalInput")
    # per-partition biases for QT/KT: col 0/1 = bq tiles, col 2/3 = bk tiles
    bias_qk = nc.dram_tensor("bias_qk", [128, 4], F32, kind="ExternalInput")
    # bv interleaved with 1.0 at each head's ones column -> [1, 260]
    bv_aug = nc.dram_tensor("bv_aug", [1, VAUG], F32, kind="ExternalInput")
    out_raw = nc.dram_tensor("out_raw", [VAUG, S], F32, kind="ExternalOutput")

    with tile.TileContext(nc) as tc:
        with (
            tc.tile_pool(name="consts", bufs=1) as consts,
            tc.tile_pool(name="work", bufs=3) as work,
            tc.tile_pool(name="outp", bufs=4) as outp,
            tc.tile_pool(name="ps_wide", bufs=2, space="PSUM") as ps_wide,
            tc.tile_pool(name="ps_ctx", bufs=4, space="PSUM") as ps_ctx,
        ):
            # ---- load inputs ----
            xT_sb = consts.tile([128, KT_TILES, S], F32)
            nc.sync.dma_start(
                xT_sb[:], xT.rearrange("(ko p) s -> p ko s", p=128)
            )
            wq_sb = consts.tile([128, KT_TILES, DG], F32)
            nc.sync.dma_start(
                wq_sb[:], wq.rearrange("(ko p) d -> p ko d", p=128)
            )
            wk_sb = consts.tile([128, KT_TILES, DG], F32)
            nc.sync.dma_start(
                wk_sb[:], wk.rearrange("(ko p) d -> p ko d", p=128)
            )
            wv_sb = consts.tile([128, KT_TILES, VAUG], F32)
            nc.sync.dma_start(
                wv_sb[:], wv.rearrange("(ko p) d -> p ko d", p=128)
            )
            biasqk_sb = consts.tile([128, 4], F32)
            nc.sync.dma_start(biasqk_sb[:], bias_qk[:])
            bvaug_sb = consts.tile([1, VAUG], F32)
            nc.sync.dma_start(bvaug_sb[:], bv_aug[:])
            ones_sb = consts.tile([1, 128], F32)
            nc.vector.memset(ones_sb[:], 1.0)

            # ---- QT / KT projections: [128, m, S], m-tile m = d' 128m.. ----
            QT_sb = consts.tile([128, 2, S], F32)
            KT_sb = consts.tile([128, 2, S], F32)
            for dst, w_sb, bias_col0 in ((QT_sb, wq_sb, 0), (KT_sb, wk_sb, 2)):
                for m in range(2):
                    for sc in range(N_QC):
                        ps = ps_wide.tile([128, 512], F32, tag="wide", name="ps_proj")
                        for kt in range(KT_TILES):
                            nc.tensor.matmul(
                                ps[:],
                                wq_sb[:, kt, m * 128:(m + 1) * 128]
                                if w_sb is wq_sb
                                else wk_sb[:, kt, m * 128:(m + 1) * 128],
                                xT_sb[:, kt, sc * QC:(sc + 1) * QC],
                                start=(kt == 0),
                                stop=(kt == KT_TILES - 1),
                            )
                        nc.vector.tensor_scalar_add(
                            dst[:, m, sc * QC:(sc + 1) * QC],
                            ps[:],
                            biasqk_sb[:, bias_col0 + m:bias_col0 + m + 1],
                        )

            # ---- V projection (head-major augmented layout) ----
            v_sb = consts.tile([128, ST_TILES, VAUG], F32)
            for st in range(ST_TILES):
                psv = ps_wide.tile([128, VAUG], F32, tag="wide", name="ps_v")
                for kt in range(KT_TILES):
                    nc.tensor.matmul(
                        psv[:],
                        xT_sb[:, kt, st * 128:(st + 1) * 128],
                        wv_sb[:, kt, :],
                        start=(kt == 0),
                        stop=False,
                    )
                # bias (and the per-head ones columns) as a rank-1 update
                nc.tensor.matmul(
                    psv[:],
                    ones_sb[:, :],
                    bvaug_sb[:, :],
                    start=False,
                    stop=True,
                )
                nc.vector.tensor_copy(out=v_sb[:, st, :], in_=psv[:])

            # ---- attention: pair p = heads (2p, 2p+1) of this group ----
            for p in range(2):
                for qc in range(N_QC):
                    ctxs = [
                        ps_ctx.tile([65, 512], F32, tag="ctx", name=f"ctx{hh}")
                        for hh in range(2)
                    ]
                    for kt in range(ST_TILES):
                        ssc = ps_wide.tile(
                            [128, 1024], F32, tag="wide", name="ps_scores"
                        )
                        for hh in range(2):  # row-packed pair
                            rows = slice(hh * 64, hh * 64 + 64)
                            nc.tensor.matmul(
                                ssc[:, hh * 512:(hh + 1) * 512],
                                KT_sb[rows, p, kt * 128:(kt + 1) * 128],
                                QT_sb[rows, p, qc * QC:(qc + 1) * QC],
                                start=True,
                                stop=True,
                            )
                        es = work.tile([128, 1024], F32, tag="es", name="es")
                        nc.scalar.activation(
                            es[:],
                            ssc[:],
                            mybir.ActivationFunctionType.Exp,
                            scale=0.125,
                        )
                        for hh in range(2):
                            h = 2 * p + hh
                            nc.tensor.matmul(
                                ctxs[hh][:],
                                v_sb[:, kt, h * 65:(h + 1) * 65],
                                es[:, hh * 512:(hh + 1) * 512],
                                start=(kt == 0),
                                stop=(kt == ST_TILES - 1),
                            )
                    for hh in range(2):
                        h = 2 * p + hh
                        ctx_sb = outp.tile([65, 512], F32, tag="o", name="ctx_sb")
                        nc.vector.tensor_copy(out=ctx_sb[:], in_=ctxs[hh][:])
                        nc.sync.dma_start(
                            out_raw[h * 65:(h + 1) * 65, qc * QC:(qc + 1) * QC],
                            ctx_sb[:],
                        )
    nc.compile()
    return nc


_NC_CACHE = None


def _get_nc():
    global _NC_CACHE
    if _NC_CACHE is None:
        _NC_CACHE = _build_kernel()
    return _NC_CACHE


def _prep_core_inputs(hidden_states, Wq, bq, Wk, bk, Wv, bv):
    """Host-side sharding: returns list of 8 in_maps."""
    xTs = [np.ascontiguousarray(hidden_states[b].T) for b in range(B)]
    in_maps = []
    for c in range(N_CORES):
        b, g = divmod(c, GROUPS)
        cs = slice(g * DG, (g + 1) * DG)
        wq_g = np.ascontiguousarray(Wq[:, cs])
        wk_g = np.ascontiguousarray(Wk[:, cs])
        wv_g = Wv[:, cs]
        bq_g, bk_g, bv_g = bq[cs], bk[cs], bv[cs]

        wv_aug = np.zeros((HIDDEN, VAUG), dtype=np.float32)
        bv_aug = np.zeros((1, VAUG), dtype=np.float32)
        for h in range(HG):
            wv_aug[:, h * 65:h * 65 + 64] = wv_g[:, h * 64:(h + 1) * 64]
            bv_aug[0, h * 65:h * 65 + 64] = bv_g[h * 64:(h + 1) * 64]
            bv_aug[0, h * 65 + 64] = 1.0

        bias_qk = np.stack(
            [bq_g[:128], bq_g[128:], bk_g[:128], bk_g[128:]], axis=1
        ).astype(np.float32)

        in_maps.append(
            {
                "xT": xTs[b],
                "wq": wq_g.astype(np.float32),
                "wk": wk_g.astype(np.float32),
                "wv": np.ascontiguousarray(wv_aug),
                "bias_qk": np.ascontiguousarray(bias_qk),
                "bv_aug": bv_aug,
            }
        )
    return in_maps


def _unshard(results):
    out = np.empty((B, S, HIDDEN), dtype=np.float32)
    for c in range(N_CORES):
        b, g = divmod(c, GROUPS)
        raw = results[c]["out_raw"]  # [260, 2048]
        for h in range(HG):
            ctx = raw[h * 65:h * 65 + 64]          # [64, S]
            sums = raw[h * 65 + 64]                # [S]
            col0 = g * DG + h * HEAD
            out[b, :, col0:col0 + HEAD] = (ctx / sums).T
    return out


def kernel(**inputs):
    inputs = {k: np.asarray(v, dtype=np.float32) for k, v in inputs.items()}
    nc = _get_nc()
    in_maps = _prep_core_inputs(**inputs)
    res = run_bass_kernel_spmd(nc, in_maps, core_ids=list(range(N_CORES)))
    return _unshard(res.results)


if __name__ == "__main__":
    rng = np.random.default_rng(0)
    scale = 1.0 / np.sqrt(HIDDEN)
    ins = {
        "hidden_states": rng.standard_normal((B, S, HIDDEN), dtype=np.float32),
        "Wq": rng.standard_normal((HIDDEN, HIDDEN), dtype=np.float32) * scale,
        "bq": rng.standard_normal(HIDDEN, dtype=np.float32) * 0.01,
        "Wk": rng.standard_normal((HIDDEN, HIDDEN), dtype=np.float32) * scale,
        "bk": rng.standard_normal(HIDDEN, dtype=np.float32) * 0.01,
        "Wv": rng.standard_normal((HIDDEN, HIDDEN), dtype=np.float32) * scale,
        "bv": rng.standard_normal(HIDDEN, dtype=np.float32) * 0.01,
    }
    out = kernel(**ins)
    # numpy reference
    def ref(x, Wq, bq, Wk, bk, Wv, bv):
        q = (x @ Wq + bq).reshape(B, S, NUM_HEADS, HEAD).transpose(0, 2, 1, 3)
        k = (x @ Wk + bk).reshape(B, S, NUM_HEADS, HEAD).transpose(0, 2, 1, 3)
        v = (x @ Wv + bv).reshape(B, S, NUM_HEADS, HEAD).transpose(0, 2, 1, 3)
        s = np.einsum("bhqd,bhkd->bhqk", q, k) / np.sqrt(HEAD)
        s = s - s.max(-1, keepdims=True)
        p = np.exp(s)
        p /= p.sum(-1, keepdims=True)
        c = np.einsum("bhqk,bhkd->bhqd", p, v)
        return c.transpose(0, 2, 1, 3).reshape(B, S, HIDDEN)

    exp = ref(
        ins["hidden_states"].astype(np.float64),
        ins["Wq"].astype(np.float64), ins["bq"].astype(np.float64),
        ins["Wk"].astype(np.float64), ins["bk"].astype(np.float64),
        ins["Wv"].astype(np.float64), ins["bv"].astype(np.float64),
    )
    err = np.abs(out - exp) / (np.abs(exp) + 1e-6)
    print("max rel err:", err.max(), "mean:", err.mean())


# revision 5
# speedup vs baseline: 2.2129x; 2.2129x over previous
"""BertSelfAttention on 8 Trainium2 NeuronCores (Bass/Tile).

Sharding: data-parallel over batch (B=2) x tensor-parallel over heads
(16 heads -> 4 groups of 4). Core c handles batch c//4, head group c%4,
holding column shards of Wq/Wk/Wv. No collectives.

All matmuls use float32r operands (TF32-class 1+8+11-bit, single PE pass,
4x the fp32 LOW_HIGH throughput; ~1.5e-4 per-matmul rel err). f32r data is
produced only by cast-DMAs (SWDGE) and ScalarE copies - the only legal
f32r producers. PSUM accumulation stays fp32 and the output path is fp32.

Per-core math (S=2048, group of 4 heads, d=64):
  xT   [1024, 2048]  host-pretransposed hidden states (this batch)
  QT/KT = W^T x^T + b (rank-1 bias matmul)   [256, 2048] f32r, d' on parts
  V     = x W_aug + b_aug                    [2048, 260] f32r, head-major
          (65 cols/head: 64 V dims + ones column -> softmax denominators)
  per pair of heads, q-chunk (512), k-tile (128):
    S^T[k,q] = sum_d KT[d,k] QT[d,q]   row-packed pairs (2x64 contraction)
    E = exp(S^T / 8)                   ScalarE, PSUM -> SBUF f32r
    ctx^T[r,q] += sum_k Vaug[k,r] E[k,q]   r=0..63 ctx, r=64 denominator
  out_raw [260, 2048] = 4 heads x (64 ctx rows + 1 sums row)

Host unshards: out[b, :, g*256 + 64h + r] = (ctx_h / sums_h).T
"""

import sys

sys.path.insert(0, "/opt/trn_rl_repo")

import numpy as np

import concourse.bass as bass
import concourse.mybir as mybir
import concourse.tile as tile
from concourse import bacc
from concourse.bass_utils import run_bass_kernel_spmd

F32 = mybir.dt.float32
F32R = mybir.dt.float32r

HIDDEN = 1024
NUM_HEADS = 16
HEAD = 64
B, S = 2, 2048
N_CORES = 8
GROUPS = 4                      # head groups (tensor parallel)
HG = NUM_HEADS // GROUPS        # heads per group = 4
DG = HG * HEAD                  # 256 cols per group
KT_TILES = HIDDEN // 128        # 8 contraction tiles for projections
ST_TILES = S // 128             # 16 sequence tiles
QC = 512                        # q chunk width
N_QC = S // QC                  # 4
VAUG = HG * (HEAD + 1)          # 260: [V_h | ones] per head


def _build_kernel():
    nc = bacc.Bacc("TRN2")

    xT = nc.dram_tensor("xT", [HIDDEN, S], F32, kind="ExternalInput")
    wq = nc.dram_tensor("wq", [HIDDEN, DG], F32, kind="ExternalInput")
    wk = nc.dram_tensor("wk", [HIDDEN, DG], F32, kind="ExternalInput")
    # wv pre-augmented on host: per head 64 cols + a zero col -> [1024, 260]
    wv = nc.dram_tensor("wv", [HIDDEN, VAUG], F32, kind="ExternalInput")
    # rank-1 bias rows: bq|bk [1, 512]; bv interleaved with 1.0 at each
    # head's ones column [1, 260]; ones [1, 512]
    bqk = nc.dram_tensor("bqk", [1, 2 * DG], F32, kind="ExternalInput")
    bv_aug = nc.dram_tensor("bv_aug", [1, VAUG], F32, kind="ExternalInput")
    ones_in = nc.dram_tensor("ones_in", [1, QC], F32, kind="ExternalInput")
    out_raw = nc.dram_tensor("out_raw", [VAUG, S], F32, kind="ExternalOutput")

    with tile.TileContext(nc) as tc:
        with (
            tc.tile_pool(name="consts", bufs=1) as consts,
            tc.tile_pool(name="work", bufs=3) as work,
            tc.tile_pool(name="outp", bufs=4) as outp,
            tc.tile_pool(name="ps_wide", bufs=2, space="PSUM") as ps_wide,
            tc.tile_pool(name="ps_ctx", bufs=4, space="PSUM") as ps_ctx,
        ):
            # ---- load inputs (cast to f32r in-flight via SWDGE) ----
            xT_sb = consts.tile([128, KT_TILES, S], F32R)
            nc.gpsimd.dma_start(
                xT_sb[:], xT.rearrange("(ko p) s -> p ko s", p=128)
            )
            wq_sb = consts.tile([128, KT_TILES, DG], F32R)
            nc.gpsimd.dma_start(
                wq_sb[:], wq.rearrange("(ko p) d -> p ko d", p=128)
            )
            wk_sb = consts.tile([128, KT_TILES, DG], F32R)
            nc.gpsimd.dma_start(
                wk_sb[:], wk.rearrange("(ko p) d -> p ko d", p=128)
            )
            wv_sb = consts.tile([128, KT_TILES, VAUG], F32R)
            nc.gpsimd.dma_start(
                wv_sb[:], wv.rearrange("(ko p) d -> p ko d", p=128)
            )
            bqk_sb = consts.tile([1, 2 * DG], F32R)
            nc.gpsimd.dma_start(bqk_sb[:], bqk[:])
            bvaug_sb = consts.tile([1, VAUG], F32R)
            nc.gpsimd.dma_start(bvaug_sb[:], bv_aug[:])
            ones_sb = consts.tile([1, QC], F32R)
            nc.gpsimd.dma_start(ones_sb[:], ones_in[:])

            # ---- QT / KT projections: [128, m, S], m-tile m = d' 128m.. ----
            QT_sb = consts.tile([128, 2, S], F32R)
            KT_sb = consts.tile([128, 2, S], F32R)
            for dst, w_sb, bcol in ((QT_sb, wq_sb, 0), (KT_sb, wk_sb, DG)):
                for m in range(2):
                    for sc in range(N_QC):
                        ps = ps_wide.tile([128, 512], F32, tag="wide", name="ps_proj")
                        for kt in range(KT_TILES):
                            nc.tensor.matmul(
                                ps[:],
                                w_sb[:, kt, m * 128:(m + 1) * 128],
                                xT_sb[:, kt, sc * QC:(sc + 1) * QC],
                                start=(kt == 0),
                                stop=False,
                            )
                        # += bias[d'] x ones[q]  (rank-1)
                        nc.tensor.matmul(
                            ps[:],
                            bqk_sb[:, bcol + m * 128:bcol + (m + 1) * 128],
                            ones_sb[:, :],
                            start=False,
                            stop=True,
                        )
                        nc.scalar.copy(
                            dst[:, m, sc * QC:(sc + 1) * QC], ps[:]
                        )

            # ---- V projection (head-major augmented layout) ----
            v_sb = consts.tile([128, ST_TILES, VAUG], F32R)
            for st in range(ST_TILES):
                psv = ps_wide.tile([128, VAUG], F32, tag="wide", name="ps_v")
                for kt in range(KT_TILES):
                    nc.tensor.matmul(
                        psv[:],
                        xT_sb[:, kt, st * 128:(st + 1) * 128],
                        wv_sb[:, kt, :],
                        start=(kt == 0),
                        stop=False,
                    )
                # bias (and the per-head ones columns) as a rank-1 update
                nc.tensor.matmul(
                    psv[:],
                    ones_sb[:, 0:128],
                    bvaug_sb[:, :],
                    start=False,
                    stop=True,
                )
                nc.scalar.copy(v_sb[:, st, :], psv[:])

            # ---- attention: pair p = heads (2p, 2p+1) of this group ----
            for p in range(2):
                for qc in range(N_QC):
                    ctxs = [
                        ps_ctx.tile([65, 512], F32, tag="ctx", name=f"ctx{hh}")
                        for hh in range(2)
                    ]
                    for kt in range(ST_TILES):
                        ssc = ps_wide.tile(
                            [128, 1024], F32, tag="wide", name="ps_scores"
                        )
                        for hh in range(2):  # row-packed pair
                            rows = slice(hh * 64, hh * 64 + 64)
                            nc.tensor.matmul(
                                ssc[:, hh * 512:(hh + 1) * 512],
                                KT_sb[rows, p, kt * 128:(kt + 1) * 128],
                                QT_sb[rows, p, qc * QC:(qc + 1) * QC],
                                start=True,
                                stop=True,
                            )
                        es = work.tile([128, 1024], F32R, tag="es", name="es")
                        nc.scalar.activation(
                            es[:],
                            ssc[:],
                            mybir.ActivationFunctionType.Exp,
                            scale=0.125,
                        )
                        for hh in range(2):
                            h = 2 * p + hh
                            nc.tensor.matmul(
                                ctxs[hh][:],
                                v_sb[:, kt, h * 65:(h + 1) * 65],
                                es[:, hh * 512:(hh + 1) * 512],
                                start=(kt == 0),
                                stop=(kt == ST_TILES - 1),
                            )
                    for hh in range(2):
                        h = 2 * p + hh
                        ctx_sb = outp.tile([65, 512], F32, tag="o", name="ctx_sb")
                        nc.vector.tensor_copy(out=ctx_sb[:], in_=ctxs[hh][:])
                        nc.sync.dma_start(
                            out_raw[h * 65:(h + 1) * 65, qc * QC:(qc + 1) * QC],
                            ctx_sb[:],
                        )
    nc.compile()
    return nc


_NC_CACHE = None


def _get_nc():
    global _NC_CACHE
    if _NC_CACHE is None:
        _NC_CACHE = _build_kernel()
    return _NC_CACHE


def _prep_core_inputs(hidden_states, Wq, bq, Wk, bk, Wv, bv):
    """Host-side sharding: returns list of 8 in_maps."""
    xTs = [np.ascontiguousarray(hidden_states[b].T) for b in range(B)]
    in_maps = []
    for c in range(N_CORES):
        b, g = divmod(c, GROUPS)
        cs = slice(g * DG, (g + 1) * DG)
        wq_g = np.ascontiguousarray(Wq[:, cs])
        wk_g = np.ascontiguousarray(Wk[:, cs])
        wv_g = Wv[:, cs]
        bq_g, bk_g, bv_g = bq[cs], bk[cs], bv[cs]

        wv_aug = np.zeros((HIDDEN, VAUG), dtype=np.float32)
        bv_aug = np.zeros((1, VAUG), dtype=np.float32)
        for h in range(HG):
            wv_aug[:, h * 65:h * 65 + 64] = wv_g[:, h * 64:(h + 1) * 64]
            bv_aug[0, h * 65:h * 65 + 64] = bv_g[h * 64:(h + 1) * 64]
            bv_aug[0, h * 65 + 64] = 1.0

        bqk = np.concatenate([bq_g, bk_g]).reshape(1, 2 * DG).astype(np.float32)

        in_maps.append(
            {
                "xT": xTs[b],
                "wq": wq_g.astype(np.float32),
                "wk": wk_g.astype(np.float32),
                "wv": np.ascontiguousarray(wv_aug),
                "bqk": np.ascontiguousarray(bqk),
                "bv_aug": bv_aug,
                "ones_in": np.ones((1, QC), dtype=np.float32),
            }
        )
    return in_maps


def _unshard(results):
    out = np.empty((B, S, HIDDEN), dtype=np.float32)
    for c in range(N_CORES):
        b, g = divmod(c, GROUPS)
        raw = results[c]["out_raw"]  # [260, 2048]
        for h in range(HG):
            ctx = raw[h * 65:h * 65 + 64]          # [64, S]
            sums = raw[h * 65 + 64]                # [S]
            col0 = g * DG + h * HEAD
            out[b, :, col0:col0 + HEAD] = (ctx / sums).T
    return out


def kernel(**inputs):
    inputs = {k: np.asarray(v, dtype=np.float32) for k, v in inputs.items()}
    nc = _get_nc()
    in_maps = _prep_core_inputs(**inputs)
    res = run_bass_kernel_spmd(nc, in_maps, core_ids=list(range(N_CORES)))
    return _unshard(res.results)


if __name__ == "__main__":
    rng = np.random.default_rng(0)
    scale = 1.0 / np.sqrt(HIDDEN)
    ins = {
        "hidden_states": rng.standard_normal((B, S, HIDDEN), dtype=np.float32),
        "Wq": rng.standard_normal((HIDDEN, HIDDEN), dtype=np.float32) * scale,
        "bq": rng.standard_normal(HIDDEN, dtype=np.float32) * 0.01,
        "Wk": rng.standard_normal((HIDDEN, HIDDEN), dtype=np.float32) * scale,
        "bk": rng.standard_normal(HIDDEN, dtype=np.float32) * 0.01,
        "Wv": rng.standard_normal((HIDDEN, HIDDEN), dtype=np.float32) * scale,
        "bv": rng.standard_normal(HIDDEN, dtype=np.float32) * 0.01,
    }
    out = kernel(**ins)

    def ref(x, Wq, bq, Wk, bk, Wv, bv):
        q = (x @ Wq + bq).reshape(B, S, NUM_HEADS, HEAD).transpose(0, 2, 1, 3)
        k = (x @ Wk + bk).reshape(B, S, NUM_HEADS, HEAD).transpose(0, 2, 1, 3)
        v = (x @ Wv + bv).reshape(B, S, NUM_HEADS, HEAD).transpose(0, 2, 1, 3)
        s = np.einsum("bhqd,bhkd->bhqk", q, k) / np.sqrt(HEAD)
        s = s - s.max(-1, keepdims=True)
        p = np.exp(s)
        p /= p.sum(-1, keepdims=True)
        c = np.einsum("bhqk,bhkd->bhqd", p, v)
        return c.transpose(0, 2, 1, 3).reshape(B, S, HIDDEN)

    exp = ref(
        ins["hidden_states"].astype(np.float64),
        ins["Wq"].astype(np.float64), ins["bq"].astype(np.float64),
        ins["Wk"].astype(np.float64), ins["bk"].astype(np.float64),
        ins["Wv"].astype(np.float64), ins["bv"].astype(np.float64),
    )
    print("L2 rel err:", np.linalg.norm(out - exp) / np.linalg.norm(exp))
    print("max abs err:", np.abs(out - exp).max())


# revision 6
# speedup vs baseline: 2.3476x; 1.0608x over previous
"""BertSelfAttention on 8 Trainium2 NeuronCores (Bass/Tile).

Sharding: data-parallel over batch (B=2) x tensor-parallel over heads
(16 heads -> 4 groups of 4). Core c handles batch c//4, head group c%4,
holding column shards of Wq/Wk/Wv. No collectives.

All matmuls use float32r operands (TF32-class 1+8+11-bit, single PE pass,
4x the fp32 LOW_HIGH throughput; ~1.5e-4 per-matmul rel err). f32r data is
produced only by cast-DMAs (SWDGE) and ScalarE copies - the only legal
f32r producers. PSUM accumulation stays fp32 and the output path is fp32.

Per-core math (S=2048, group of 4 heads, d=64):
  xT   [1024, 2048]  host-pretransposed hidden states (this batch)
  QT/KT = W^T x^T + b (rank-1 bias matmul)   [256, 2048] f32r, d' on parts
  V     = x W_aug + b_aug                    [2048, 260] f32r, head-major
          (65 cols/head: 64 V dims + ones column -> softmax denominators)
  per pair of heads, q-chunk (512), k-tile (128):
    S^T[k,q] = sum_d KT[d,k] QT[d,q]   row-packed pairs (2x64 contraction)
    E = exp(S^T / 8)                   ScalarE, PSUM -> SBUF f32r
    ctx^T[r,q] += sum_k Vaug[k,r] E[k,q]   r=0..63 ctx, r=64 denominator
  out_raw [260, 2048] = 4 heads x (64 ctx rows + 1 sums row)

Host unshards: out[b, :, g*256 + 64h + r] = (ctx_h / sums_h).T
"""

import sys

sys.path.insert(0, "/opt/trn_rl_repo")

import numpy as np

import concourse.bass as bass
import concourse.mybir as mybir
import concourse.tile as tile
from concourse import bacc
from concourse.bass_utils import run_bass_kernel_spmd

F32 = mybir.dt.float32
F32R = mybir.dt.float32r

HIDDEN = 1024
NUM_HEADS = 16
HEAD = 64
B, S = 2, 2048
N_CORES = 8
GROUPS = 4                      # head groups (tensor parallel)
HG = NUM_HEADS // GROUPS        # heads per group = 4
DG = HG * HEAD                  # 256 cols per group
KT_TILES = HIDDEN // 128        # 8 contraction tiles for projections
ST_TILES = S // 128             # 16 sequence tiles
QC = 512                        # q chunk width
N_QC = S // QC                  # 4
VAUG = HG * (HEAD + 1)          # 260: [V_h | ones] per head


def _build_kernel():
    nc = bacc.Bacc("TRN2")

    xT = nc.dram_tensor("xT", [HIDDEN, S], F32, kind="ExternalInput")
    wq = nc.dram_tensor("wq", [HIDDEN, DG], F32, kind="ExternalInput")
    wk = nc.dram_tensor("wk", [HIDDEN, DG], F32, kind="ExternalInput")
    # wv pre-augmented on host: per head 64 cols + a zero col -> [1024, 260]
    wv = nc.dram_tensor("wv", [HIDDEN, VAUG], F32, kind="ExternalInput")
    # rank-1 bias rows: bq|bk [1, 512]; bv interleaved with 1.0 at each
    # head's ones column [1, 260]; ones [1, 512]
    bqk = nc.dram_tensor("bqk", [1, 2 * DG], F32, kind="ExternalInput")
    bv_aug = nc.dram_tensor("bv_aug", [1, VAUG], F32, kind="ExternalInput")
    ones_in = nc.dram_tensor("ones_in", [1, QC], F32, kind="ExternalInput")
    out_raw = nc.dram_tensor("out_raw", [VAUG, S], F32, kind="ExternalOutput")

    with tile.TileContext(nc) as tc:
        with (
            tc.tile_pool(name="consts", bufs=1) as consts,
            tc.tile_pool(name="work", bufs=4) as work,
            tc.tile_pool(name="outp", bufs=4) as outp,
            # 4 one-bank slots: QK-projection accumulators, then ctx accums
            tc.tile_pool(name="ps_b1", bufs=4, space="PSUM") as ps_b1,
            # 2 two-bank slots: V-projection psum, then score tiles
            tc.tile_pool(name="ps_b2", bufs=2, space="PSUM") as ps_b2,
        ):
            # ---- load inputs (cast to f32r in-flight via SWDGE) ----
            wq_sb = consts.tile([128, KT_TILES, DG], F32R)
            nc.gpsimd.dma_start(
                wq_sb[:], wq.rearrange("(ko p) d -> p ko d", p=128)
            )
            wk_sb = consts.tile([128, KT_TILES, DG], F32R)
            nc.gpsimd.dma_start(
                wk_sb[:], wk.rearrange("(ko p) d -> p ko d", p=128)
            )
            wv_sb = consts.tile([128, KT_TILES, VAUG], F32R)
            nc.gpsimd.dma_start(
                wv_sb[:], wv.rearrange("(ko p) d -> p ko d", p=128)
            )
            bqk_sb = consts.tile([1, 2 * DG], F32R)
            nc.gpsimd.dma_start(bqk_sb[:], bqk[:])
            bvaug_sb = consts.tile([1, VAUG], F32R)
            nc.gpsimd.dma_start(bvaug_sb[:], bv_aug[:])
            ones_sb = consts.tile([1, QC], F32R)
            nc.gpsimd.dma_start(ones_sb[:], ones_in[:])
            # xT in per-kt chunks so compute can start early
            xT_sb = consts.tile([128, KT_TILES, S], F32R)
            xT_r = xT.rearrange("(ko p) s -> p ko s", p=128)
            for kt in range(KT_TILES):
                nc.gpsimd.dma_start(xT_sb[:, kt, :], xT_r[:, kt, :])

            # ---- QT / KT projections: [128, m, S], m-tile m = d' 128m.. ----
            # kt-outer / sc-inner so each weight tile is loaded once into the
            # PE and reused across the 4 moving chunks.
            QT_sb = consts.tile([128, 2, S], F32R)
            KT_sb = consts.tile([128, 2, S], F32R)
            for m in range(2):
                for dst, w_sb, bcol in ((QT_sb, wq_sb, 0), (KT_sb, wk_sb, DG)):
                    pss = [
                        ps_b1.tile([128, 512], F32, tag="b1", name=f"ps_proj{sc}")
                        for sc in range(N_QC)
                    ]
                    for kt in range(KT_TILES):
                        for sc in range(N_QC):
                            nc.tensor.matmul(
                                pss[sc][:],
                                w_sb[:, kt, m * 128:(m + 1) * 128],
                                xT_sb[:, kt, sc * QC:(sc + 1) * QC],
                                start=(kt == 0),
                                stop=False,
                            )
                    for sc in range(N_QC):
                        # += bias[d'] x ones[q]  (rank-1)
                        nc.tensor.matmul(
                            pss[sc][:],
                            bqk_sb[:, bcol + m * 128:bcol + (m + 1) * 128],
                            ones_sb[:, :],
                            start=False,
                            stop=True,
                        )
                        nc.scalar.copy(
                            dst[:, m, sc * QC:(sc + 1) * QC], pss[sc][:]
                        )

            # ---- V projection (head-major augmented layout) ----
            v_sb = consts.tile([128, ST_TILES, VAUG], F32R)
            for st in range(ST_TILES):
                psv = ps_b2.tile([128, 1024], F32, tag="b2", name="ps_v")
                for kt in range(KT_TILES):
                    nc.tensor.matmul(
                        psv[:, 0:VAUG],
                        xT_sb[:, kt, st * 128:(st + 1) * 128],
                        wv_sb[:, kt, :],
                        start=(kt == 0),
                        stop=False,
                    )
                # bias (and the per-head ones columns) as a rank-1 update
                nc.tensor.matmul(
                    psv[:, 0:VAUG],
                    ones_sb[:, 0:128],
                    bvaug_sb[:, :],
                    start=False,
                    stop=True,
                )
                nc.scalar.copy(v_sb[:, st, :], psv[:, 0:VAUG])

            # ---- attention: pair p = heads (2p, 2p+1) of this group ----
            # Two q-chunks per pass so each stationary (KT slice / V slice)
            # serves two moving chunks -> half the LDWEIGHTS.
            for p in range(2):
                for qq in range(N_QC // 2):
                    qcs = (2 * qq, 2 * qq + 1)
                    ctxs = {
                        (j, hh): ps_b1.tile(
                            [65, 512], F32, tag="b1", name=f"ctx{j}{hh}"
                        )
                        for j in range(2)
                        for hh in range(2)
                    }
                    for kt in range(ST_TILES):
                        sscs = [
                            ps_b2.tile([128, 1024], F32, tag="b2", name=f"ssc{j}")
                            for j in range(2)
                        ]
                        for hh in range(2):  # same KT slice for both chunks
                            rows = slice(hh * 64, hh * 64 + 64)
                            for j, qc in enumerate(qcs):
                                nc.tensor.matmul(
                                    sscs[j][:, hh * 512:(hh + 1) * 512],
                                    KT_sb[rows, p, kt * 128:(kt + 1) * 128],
                                    QT_sb[rows, p, qc * QC:(qc + 1) * QC],
                                    start=True,
                                    stop=True,
                                )
                        ess = []
                        for j in range(2):
                            es = work.tile(
                                [128, 1024], F32R, tag="es", name=f"es{j}"
                            )
                            nc.scalar.activation(
                                es[:],
                                sscs[j][:],
                                mybir.ActivationFunctionType.Exp,
                                scale=0.125,
                            )
                            ess.append(es)
                        for hh in range(2):  # same V slice for both chunks
                            h = 2 * p + hh
                            for j in range(2):
                                nc.tensor.matmul(
                                    ctxs[(j, hh)][:],
                                    v_sb[:, kt, h * 65:(h + 1) * 65],
                                    ess[j][:, hh * 512:(hh + 1) * 512],
                                    start=(kt == 0),
                                    stop=(kt == ST_TILES - 1),
                                )
                    for j, qc in enumerate(qcs):
                        for hh in range(2):
                            h = 2 * p + hh
                            ctx_sb = outp.tile(
                                [65, 512], F32, tag="o", name="ctx_sb"
                            )
                            nc.vector.tensor_copy(
                                out=ctx_sb[:], in_=ctxs[(j, hh)][:]
                            )
                            nc.sync.dma_start(
                                out_raw[
                                    h * 65:(h + 1) * 65,
                                    qc * QC:(qc + 1) * QC,
                                ],
                                ctx_sb[:],
                            )
    nc.compile()
    return nc


_NC_CACHE = None


def _get_nc():
    global _NC_CACHE
    if _NC_CACHE is None:
        _NC_CACHE = _build_kernel()
    return _NC_CACHE


def _prep_core_inputs(hidden_states, Wq, bq, Wk, bk, Wv, bv):
    """Host-side sharding: returns list of 8 in_maps."""
    xTs = [np.ascontiguousarray(hidden_states[b].T) for b in range(B)]
    in_maps = []
    for c in range(N_CORES):
        b, g = divmod(c, GROUPS)
        cs = slice(g * DG, (g + 1) * DG)
        wq_g = np.ascontiguousarray(Wq[:, cs])
        wk_g = np.ascontiguousarray(Wk[:, cs])
        wv_g = Wv[:, cs]
        bq_g, bk_g, bv_g = bq[cs], bk[cs], bv[cs]

        wv_aug = np.zeros((HIDDEN, VAUG), dtype=np.float32)
        bv_aug = np.zeros((1, VAUG), dtype=np.float32)
        for h in range(HG):
            wv_aug[:, h * 65:h * 65 + 64] = wv_g[:, h * 64:(h + 1) * 64]
            bv_aug[0, h * 65:h * 65 + 64] = bv_g[h * 64:(h + 1) * 64]
            bv_aug[0, h * 65 + 64] = 1.0

        bqk = np.concatenate([bq_g, bk_g]).reshape(1, 2 * DG).astype(np.float32)

        in_maps.append(
            {
                "xT": xTs[b],
                "wq": wq_g.astype(np.float32),
                "wk": wk_g.astype(np.float32),
                "wv": np.ascontiguousarray(wv_aug),
                "bqk": np.ascontiguousarray(bqk),
                "bv_aug": bv_aug,
                "ones_in": np.ones((1, QC), dtype=np.float32),
            }
        )
    return in_maps


def _unshard(results):
    out = np.empty((B, S, HIDDEN), dtype=np.float32)
    for c in range(N_CORES):
        b, g = divmod(c, GROUPS)
        raw = results[c]["out_raw"]  # [260, 2048]
        for h in range(HG):
            ctx = raw[h * 65:h * 65 + 64]          # [64, S]
            sums = raw[h * 65 + 64]                # [S]
            col0 = g * DG + h * HEAD
            out[b, :, col0:col0 + HEAD] = (ctx / sums).T
    return out


def kernel(**inputs):
    inputs = {k: np.asarray(v, dtype=np.float32) for k, v in inputs.items()}
    nc = _get_nc()
    in_maps = _prep_core_inputs(**inputs)
    res = run_bass_kernel_spmd(nc, in_maps, core_ids=list(range(N_CORES)))
    return _unshard(res.results)


if __name__ == "__main__":
    rng = np.random.default_rng(0)
    scale = 1.0 / np.sqrt(HIDDEN)
    ins = {
        "hidden_states": rng.standard_normal((B, S, HIDDEN), dtype=np.float32),
        "Wq": rng.standard_normal((HIDDEN, HIDDEN), dtype=np.float32) * scale,
        "bq": rng.standard_normal(HIDDEN, dtype=np.float32) * 0.01,
        "Wk": rng.standard_normal((HIDDEN, HIDDEN), dtype=np.float32) * scale,
        "bk": rng.standard_normal(HIDDEN, dtype=np.float32) * 0.01,
        "Wv": rng.standard_normal((HIDDEN, HIDDEN), dtype=np.float32) * scale,
        "bv": rng.standard_normal(HIDDEN, dtype=np.float32) * 0.01,
    }
    out = kernel(**ins)

    def ref(x, Wq, bq, Wk, bk, Wv, bv):
        q = (x @ Wq + bq).reshape(B, S, NUM_HEADS, HEAD).transpose(0, 2, 1, 3)
        k = (x @ Wk + bk).reshape(B, S, NUM_HEADS, HEAD).transpose(0, 2, 1, 3)
        v = (x @ Wv + bv).reshape(B, S, NUM_HEADS, HEAD).transpose(0, 2, 1, 3)
        s = np.einsum("bhqd,bhkd->bhqk", q, k) / np.sqrt(HEAD)
        s = s - s.max(-1, keepdims=True)
        p = np.exp(s)
        p /= p.sum(-1, keepdims=True)
        c = np.einsum("bhqk,bhkd->bhqd", p, v)
        return c.transpose(0, 2, 1, 3).reshape(B, S, HIDDEN)

    exp = ref(
        ins["hidden_states"].astype(np.float64),
        ins["Wq"].astype(np.float64), ins["bq"].astype(np.float64),
        ins["Wk"].astype(np.float64), ins["bk"].astype(np.float64),
        ins["Wv"].astype(np.float64), ins["bv"].astype(np.float64),
    )
    print("L2 rel err:", np.linalg.norm(out - exp) / np.linalg.norm(exp))
    print("max abs err:", np.abs(out - exp).max())
